# revision 39
# baseline (speedup 1.0000x reference)
"""Trainium2 Bass kernel for nn_DeformableConvStandard.

The deformable interpolation + both convs are linear in `inp` once the
(tiny) offsets are known, so the whole module collapses to

    out = Pt + Pd * sigmoid(ctrl' @ W),   Pt = X @ A_t,  Pd = X @ D

with A_t, D: [48, 12] host-built from offsets/conv weights, and the gate
bias pre-folded into ctrl' = ctrl + bparam @ W^-1 on the host. Rows of
[A_t|D] that are identically zero (deform positions never sampled) are
pruned from both the weights and the streamed X features, shrinking the
dominant DMA stream.

Feature-major layout: one "supertile" = one batch = 4096 rows = 8 groups
of 512 nodes. Matmuls contract 4*chunk partitions (chunk = half the used
features) and stream 512 columns. All input DMAs are hoisted and issued
on the sync queue with fully pre-allocated SBUF tiles (no ring-reuse
waits), so the exclusive DMA engine never stalls on compute. The
per-(supertile,half) combine out = Pt + Pd*S is spread across DVE
(mults), PE identity-accumulate + ACT copy (half the adds), and the
otherwise-idle Pool/GPSIMD engine (other half).
"""
import numpy as np
from contextlib import ExitStack

N_CORES = 8
B, NNODES = 128, 4096
NDW, LL, NPRED, NDRIFT = 3, 16, 12, 2
BPC = B // N_CORES          # batches per core: 16
NG = NNODES // 512          # node groups per batch: 8
STREAM_FP16 = True          # stream xp/ct/weights as fp16 (halves DMA)
OUT_FP16 = True             # store outputs as fp16 (half out-DMA)
MAX_W_COND = 1e4            # fold bias into ctrl only if W is this well-posed

# engine for each (supertile-in-pair, half) add: "pe" = identity-matmul
# accumulate + ACT copy-out; "dma" = identity-matmul accumulate + direct f32
# DMA from PSUM (no ACT); "dve" = DVE add
ADD_ENGINES = ("pe", "pe", "pe", "dve")
OUT_DMA_ENGINE = "sync"     # engine queue for output stores (inputs are hoisted)
FINE_XP_PAIRS = (0,)        # pairs with per-supertile xp DMAs (faster start)
SPLIT_LAST_OUT = False      # last pair: store per supertile (shorter tail)
N_WARM_MM = 2               # dummy PE warm-up matmuls


def _build_A(offset, conv_w, mode):
    """A [48, 12] with pred = X @ A for X [rows, 48], feature = d*16+l."""
    off = np.asarray(offset, np.float32)
    pos = np.tanh(off) * np.float32(NDRIFT) + (
        np.arange(NPRED, dtype=np.float32) + np.float32(NDRIFT)
    )[None, :]
    key = np.floor(pos)
    frac = (pos - key).astype(np.float64)
    idx = key.astype(np.int32)
    M = np.zeros((NDW, LL, NPRED), np.float64)
    for d in range(NDW):
        for j in range(NPRED):
            M[d, idx[d, j], j] += 1.0 - frac[d, j]
            M[d, idx[d, j] + 1, j] += frac[d, j]
    A = np.zeros((NDW, LL, NPRED), np.float64)
    w = np.asarray(conv_w, np.float64)
    if mode == "t":
        for p in range(NPRED):
            for k in range(3):
                j = p + k - 1
                if 0 <= j < NPRED:
                    A[:, :, p] += w[0, :, k][:, None] * M[:, :, j]
    else:
        for o in range(NPRED):
            for d in range(NDW):
                for c in range(NPRED):
                    A[d, :, o] += w[o, c, d] * M[d, :, c]
    return A.reshape(NDW * LL, NPRED)


def _plan(offset_t, offset_n, conv_t_w, conv_n_w, W):
    """Returns (order [2*chunk feature indices], chunk, fold_bias, A_t, D)."""
    A_t = _build_A(offset_t, conv_t_w, "t")
    D = _build_A(offset_n, conv_n_w, "n") - A_t
    used = (np.abs(A_t).max(axis=1) > 0) | (np.abs(D).max(axis=1) > 0)
    idx = np.where(used)[0]
    chunk = (len(idx) + 1) // 2
    order = np.concatenate([idx, np.zeros(2 * chunk - len(idx), np.int64)])
    Wf = np.asarray(W, np.float64)
    fold = np.linalg.cond(Wf) < MAX_W_COND
    return order, chunk, fold, A_t, D


def _build_weight_tiles(order, chunk, fold, A_t, D, W):
    """Stationary lhsT tiles [NW, 96, 112]: [0]=blockdiag-W (+bias perm if
    not folded at [NW-1]), [1],[2]=X chunks, [3]=I48."""
    nw = 4 if fold else 5
    wts = np.zeros((nw, 96, 112), np.float64)
    Wf = np.asarray(W, np.float64)
    for j in range(8):
        col = (j // 4) * 64 + (j % 4) * 12
        wts[0, j * 12:(j + 1) * 12, col:col + 12] = Wf
        if not fold:
            for q in range(12):
                wts[4, j * 12 + q, col + q] = 1.0
    wts[3, 0:48, 0:48] = np.eye(48)
    A_u = A_t[order]   # [2*chunk, 12]; padded rows get zero weight below
    D_u = D[order]
    pad = np.ones(2 * chunk)
    pad[len(np.where((np.abs(A_t).max(1) > 0) | (np.abs(D).max(1) > 0))[0]):] = 0
    A_u = A_u * pad[:, None]
    D_u = D_u * pad[:, None]
    for c in range(2):
        for g in range(4):
            rows = slice(g * chunk, (g + 1) * chunk)
            wts[1 + c, rows, g * 12:(g + 1) * 12] = A_u[c * chunk:(c + 1) * chunk]
            wts[1 + c, rows, 64 + g * 12:64 + (g + 1) * 12] = D_u[c * chunk:(c + 1) * chunk]
    return wts.astype(np.float32)


def build_program(chunk=20, fold=True):
    import concourse.bass as bass
    import concourse.tile as tile
    from concourse import bacc, mybir
    from concourse.bass_interp import get_hw_module

    dt_mm = mybir.dt.float16 if STREAM_FP16 else mybir.dt.float32
    f32 = mybir.dt.float32
    dt_out = mybir.dt.float16 if OUT_FP16 else f32
    NW = 4 if fold else 5
    CP = 4 * chunk              # contract partitions for the x path

    nc = bacc.Bacc("TRN2", target_bir_lowering=False, debug=False,
                   num_devices=N_CORES)
    xp = nc.dram_tensor("xp", [BPC, 2, 2, CP, 512], dt_mm, kind="ExternalInput").ap()
    ct = nc.dram_tensor("ct", [BPC, 96, 512], dt_mm, kind="ExternalInput").ap()
    wts = nc.dram_tensor("wts", [NW, 96, 112], dt_mm, kind="ExternalInput").ap()
    if not fold:
        bias = nc.dram_tensor("bias", [96, 512], dt_mm, kind="ExternalInput").ap()
    yp = nc.dram_tensor("yp", [BPC, 96, 512], dt_out, kind="ExternalOutput").ap()
    n_dma_units = sum(e == "dma" for e in ADD_ENGINES)
    if n_dma_units:
        yq = nc.dram_tensor("yq", [8, n_dma_units, 48, 512], f32,
                            kind="ExternalOutput").ap()

    with tile.TileContext(nc) as tc, ExitStack() as ctx:
        consts = ctx.enter_context(tc.tile_pool(name="consts", bufs=1))
        xpool = ctx.enter_context(tc.tile_pool(name="xp", bufs=1))
        cpool = ctx.enter_context(tc.tile_pool(name="ct", bufs=1))
        spool = ctx.enter_context(tc.tile_pool(name="sig", bufs=6))
        tpool = ctx.enter_context(tc.tile_pool(name="tmp", bufs=10))
        opool = ctx.enter_context(tc.tile_pool(name="ost", bufs=1))
        gps = ctx.enter_context(
            tc.tile_pool(name="gps", bufs=1, space=bass.MemorySpace.PSUM))
        xps = ctx.enter_context(
            tc.tile_pool(name="xps", bufs=6, space=bass.MemorySpace.PSUM))

        # ---- all input DMAs up-front, in pipeline order, no ring waits ----
        w_sb = consts.tile([96, NW * 112], dt_mm)
        nc.sync.dma_start(
            w_sb[:].rearrange("p (n f) -> p n f", n=NW),
            wts[:].rearrange("n p f -> p n f"),
        )
        if not fold:
            bias_sb = consts.tile([96, 512], dt_mm)
            nc.sync.dma_start(bias_sb[:], bias[:])

        ct_tiles = []
        xp_tiles = [None] * 8   # per pair: tile or (tile, tile) when fine

        def make_ct(i4):
            ct_sb = cpool.tile([96, 4 * 512], dt_mm, name=f"ct{i4}")
            ct_tiles.append(ct_sb)
            return ct_sb

        def load_ct_half(i4, half):
            # 2-supertile ct pieces: pair (2*i4+half)'s gate data lands just
            # before its xp pair, halving the gate-path fill latency
            c0 = i4 * 4 + half * 2
            nc.sync.dma_start(
                ct_tiles[i4][:, half * 1024:(half + 1) * 1024]
                .rearrange("p (b f) -> p b f", b=2),
                ct[c0:c0 + 2].rearrange("b p f -> p b f"),
            )

        def load_xp(pair):
            b0 = pair * 2
            if pair in FINE_XP_PAIRS:
                tl = []
                for bi in range(2):
                    t1 = xpool.tile([CP, 4 * 512], dt_mm, name=f"xf{pair}_{bi}")
                    nc.sync.dma_start(
                        t1[:].rearrange("p (h c f) -> p h c f", h=2, c=2),
                        xp[b0 + bi].rearrange("h c p f -> p h c f"),
                    )
                    tl.append(t1)
                xp_tiles[pair] = tuple(tl)
            else:
                t2 = xpool.tile([CP, 2 * 4 * 512], dt_mm, name=f"xp{pair}")
                nc.sync.dma_start(
                    t2[:].rearrange("p (b h c f) -> p b h c f", b=2, h=2, c=2),
                    xp[b0:b0 + 2].rearrange("b h c p f -> p b h c f"),
                )
                xp_tiles[pair] = t2

        for i4 in range(4):
            make_ct(i4)
        for pair in range(8):
            load_ct_half(pair // 2, pair % 2)
            load_xp(pair)

        def w_slice(k, width=112):
            return w_sb[:, k * 112:k * 112 + width]

        if N_WARM_MM:
            # tiny warm-up matmuls: near-free, but bump the PE out of its
            # cold p-state before the first real gate matmul
            wp = xps.tile([112, 512], f32, name="warmps", tag="px")
            for i in range(N_WARM_MM):
                nc.tensor.matmul(wp[0:16, 0:16], w_sb[:, 0:16],
                                 w_sb[:, 0:16], start=True, stop=True)

        # ---- compute: gate/sigmoid stage runs one pair ahead of x/combine ----
        def gates_stage(pair):
            i4, half = divmod(pair, 2)
            ct_sb = ct_tiles[i4]
            # both supertiles' gates in one 2-bank PSUM tile -> ONE sigmoid
            g2 = gps.tile([112, 2 * 512], f32)
            for bi in range(2):
                bb = half * 2 + bi
                nc.tensor.matmul(g2[:, bi * 512:(bi + 1) * 512], w_slice(0),
                                 ct_sb[:, bb * 512:(bb + 1) * 512],
                                 start=True, stop=fold)
            if not fold:
                for bi in range(2):
                    nc.tensor.matmul(g2[:, bi * 512:(bi + 1) * 512],
                                     w_slice(NW - 1), bias_sb[:],
                                     start=False, stop=True)
            s2 = spool.tile([112, 2 * 512], f32)
            nc.scalar.activation(
                s2[:], g2[:], mybir.ActivationFunctionType.Sigmoid)
            return s2

        def x_combine_stage(pair, s2):
            b0 = pair * 2
            o_sb = opool.tile([112, 2 * 512], dt_out, name=f"o{pair}")
            # x-path matmuls: one pass produces [Pt | pad | Pd] per (bi, h)
            px_list = [xps.tile([112, 512], f32, name=f"px{k}", tag="px")
                       for k in range(4)]
            for bi in range(2):
                for ci in range(2):
                    for h in range(2):
                        k = bi * 2 + h
                        if pair in FINE_XP_PAIRS:
                            rhs = xp_tiles[pair][bi][:, h * 1024 + ci * 512:
                                                     h * 1024 + (ci + 1) * 512]
                        else:
                            rhs = xp_tiles[pair][:, k * 1024 + ci * 512:
                                                 k * 1024 + (ci + 1) * 512]
                        nc.tensor.matmul(px_list[k][:],
                                         w_sb[0:CP, (1 + ci) * 112:(2 + ci) * 112],
                                         rhs, start=(ci == 0), stop=(ci == 1))
            # combine: o = Pt + Pd * S, spread across DVE / PE+ACT / PSUM-DMA
            out_eng = getattr(nc, OUT_DMA_ENGINE)
            dma_j = 0
            for k in range(4):
                bi, h = divmod(k, 2)
                px = px_list[k]
                o_slice = o_sb[64 * h:64 * h + 48, bi * 512:(bi + 1) * 512]
                t_sb = tpool.tile([48, 512], dt_mm)
                nc.vector.tensor_mul(
                    t_sb[:], px[64:112, :],
                    s2[64 * h:64 * h + 48, bi * 512:(bi + 1) * 512])
                eng = ADD_ENGINES[k]
                if k == 3 and pair in (3, 7):
                    eng = "pe"   # rebalance: ACT slack from merged sigmoid
                if eng in ("pe", "dma"):
                    # out = Pt + T via identity-matmul accumulate on PE
                    nc.tensor.matmul(px[0:48, :],
                                     w_sb[0:48, 3 * 112:3 * 112 + 48],
                                     t_sb[:], start=False, stop=True,
                                     skip_group_check=True)
                    if eng == "pe":
                        nc.scalar.activation(
                            o_slice, px[0:48, :],
                            mybir.ActivationFunctionType.Copy)
                    else:
                        # store the finished rows straight from PSUM (f32)
                        out_eng.dma_start(yq[pair, dma_j], px[0:48, :])
                        dma_j += 1
                else:
                    nc.vector.tensor_add(o_slice, px[0:48, :], t_sb[:])
            # fp16 output stores (compute-gated)
            b0 = pair * 2
            if SPLIT_LAST_OUT and pair == 7:
                for h in range(2):
                    for bi in range(2):
                        out_eng.dma_start(
                            yp[b0 + bi, 48 * h:48 * h + 48],
                            o_sb[64 * h:64 * h + 48, bi * 512:(bi + 1) * 512])
            else:
                out_eng.dma_start(
                    yp[b0:b0 + 2, 0:48].rearrange("b p f -> p b f"),
                    o_sb[0:48, :].rearrange("p (b f) -> p b f", b=2))
                out_eng.dma_start(
                    yp[b0:b0 + 2, 48:96].rearrange("b p f -> p b f"),
                    o_sb[64:112, :].rearrange("p (b f) -> p b f", b=2))

        s_store = {}
        for p in range(9):
            if p < 8:
                s_store[p] = gates_stage(p)
            if p >= 1:
                x_combine_stage(p - 1, s_store.pop(p - 1))

    nc.compile()
    nc.m = get_hw_module(nc.m)
    return nc


_PROGRAMS = {}


def _get_program(chunk, fold):
    key = (chunk, fold)
    if key not in _PROGRAMS:
        _PROGRAMS[key] = build_program(chunk, fold)
    return _PROGRAMS[key]


def pack_inputs(inp, ctrl, bparam, W, order, chunk, fold):
    """Host-side shard + layout packing. Returns in_maps (list of 8 dicts)."""
    X = np.asarray(inp, np.float32).reshape(B, 2, 4, 512, NDW * LL)
    Xu = X[..., order].reshape(B, 2, 4, 512, 2, chunk)
    Xpack = np.ascontiguousarray(Xu.transpose(0, 1, 4, 2, 5, 3)).reshape(
        B, 2, 2, 4 * chunk, 512)
    ctf = np.asarray(ctrl, np.float64)
    if fold:
        binv = np.asarray(bparam, np.float64) @ np.linalg.inv(
            np.asarray(W, np.float64))                      # [NNODES, 12]
        ctf = ctf + binv[None, :, :]
    CT = np.ascontiguousarray(
        ctf.astype(np.float32).reshape(B, NG, 512, 12).transpose(0, 1, 3, 2)
    ).reshape(B, 96, 512)
    dt = np.float16 if STREAM_FP16 else np.float32
    Xpack = Xpack.astype(dt)
    CT = CT.astype(dt)
    in_maps = []
    for c in range(N_CORES):
        sl = slice(c * BPC, (c + 1) * BPC)
        in_maps.append({"xp": Xpack[sl], "ct": CT[sl]})
    return in_maps


def unpack_output(results):
    """results: list of 8 dicts with 'yp' [BPC, 96, 512] (+ 'yq' f32 direct
    slices) -> out [B, N, 12]."""
    yp = np.concatenate([r["yp"].astype(np.float32) for r in results], axis=0)
    return np.ascontiguousarray(
        yp.reshape(B, NG, 12, 512).transpose(0, 1, 3, 2)
    ).reshape(B, NNODES, NPRED)


def kernel(inp, ctrl, offset_t, offset_n, conv_t_w, conv_t_b, conv_n_w,
           conv_n_b, W, bparam):
    from concourse.bass_utils import run_bass_kernel_spmd

    order, chunk, fold, A_t, D = _plan(offset_t, offset_n, conv_t_w,
                                       conv_n_w, W)
    nc = _get_program(chunk, fold)
    wts_np = _build_weight_tiles(order, chunk, fold, A_t, D, W)
    dt = np.float16 if STREAM_FP16 else np.float32
    in_maps = pack_inputs(inp, ctrl, bparam, W, order, chunk, fold)
    for m in in_maps:
        m["wts"] = wts_np.astype(dt)
        if not fold:
            bias_t = np.ascontiguousarray(
                np.asarray(bparam, np.float32).reshape(NG, 512, 12)
                .transpose(0, 2, 1)).reshape(96, 512)
            m["bias"] = bias_t.astype(dt)
    res = run_bass_kernel_spmd(nc, in_maps, core_ids=list(range(N_CORES)))
    out = unpack_output(res.results)
    # Conv biases are zeros in this module's init, so the device kernel omits
    # them. If ever nonzero, apply the exact correction on the host.
    ctb = float(np.asarray(conv_t_b).reshape(-1)[0])
    cnb = np.asarray(conv_n_b, np.float32)
    if ctb != 0.0 or np.any(cnb != 0.0):
        G = np.asarray(ctrl, np.float32).reshape(B * NNODES, NPRED) @ np.asarray(
            W, np.float32)
        G += np.tile(np.asarray(bparam, np.float32), (B, 1))
        S = 1.0 / (1.0 + np.exp(-G))
        out = out + (ctb + (cnb[None, :] - ctb) * S).reshape(B, NNODES, NPRED)
    return out.astype(np.float32)


# revision 40
# speedup vs baseline: 1.0095x; 1.0095x over previous
"""Trainium2 Bass kernel for nn_DeformableConvStandard.

The deformable interpolation + both convs are linear in `inp` once the
(tiny) offsets are known, so the whole module collapses to

    out = Pt + Pd * sigmoid(ctrl' @ W),   Pt = X @ A_t,  Pd = X @ D

with A_t, D: [48, 12] host-built from offsets/conv weights, and the gate
bias pre-folded into ctrl' = ctrl + bparam @ W^-1 on the host. Rows of
[A_t|D] that are identically zero (deform positions never sampled) are
pruned from both the weights and the streamed X features, shrinking the
dominant DMA stream.

Feature-major layout: one "supertile" = one batch = 4096 rows = 8 groups
of 512 nodes. Matmuls contract 4*chunk partitions (chunk = half the used
features) and stream 512 columns. All input DMAs are hoisted and issued
on the sync queue with fully pre-allocated SBUF tiles (no ring-reuse
waits), so the exclusive DMA engine never stalls on compute. The
per-(supertile,half) combine out = Pt + Pd*S is spread across DVE
(mults), PE identity-accumulate + ACT copy (half the adds), and the
otherwise-idle Pool/GPSIMD engine (other half).
"""
import numpy as np
from contextlib import ExitStack

N_CORES = 8
B, NNODES = 128, 4096
NDW, LL, NPRED, NDRIFT = 3, 16, 12, 2
BPC = B // N_CORES          # batches per core: 16
NG = NNODES // 512          # node groups per batch: 8
STREAM_FP16 = True          # stream xp/ct/weights as fp16 (halves DMA)
OUT_FP16 = True             # store outputs as fp16 (half out-DMA)
MAX_W_COND = 1e4            # fold bias into ctrl only if W is this well-posed

# engine for each (supertile-in-pair, half) add: "pe" = identity-matmul
# accumulate + ACT copy-out; "dma" = identity-matmul accumulate + direct f32
# DMA from PSUM (no ACT); "dve" = DVE add
ADD_ENGINES = ("pe", "pe", "pe", "dve")
OUT_DMA_ENGINE = "sync"     # engine queue for output stores (inputs are hoisted)
FINE_XP_PAIRS = (0,)        # pairs with per-supertile xp DMAs (faster start)
SPLIT_LAST_OUT = False      # last pair: store per supertile (shorter tail)
N_WARM_MM = 2               # dummy PE warm-up matmuls


def _build_A(offset, conv_w, mode):
    """A [48, 12] with pred = X @ A for X [rows, 48], feature = d*16+l."""
    off = np.asarray(offset, np.float32)
    pos = np.tanh(off) * np.float32(NDRIFT) + (
        np.arange(NPRED, dtype=np.float32) + np.float32(NDRIFT)
    )[None, :]
    key = np.floor(pos)
    frac = (pos - key).astype(np.float64)
    idx = key.astype(np.int32)
    M = np.zeros((NDW, LL, NPRED), np.float64)
    for d in range(NDW):
        for j in range(NPRED):
            M[d, idx[d, j], j] += 1.0 - frac[d, j]
            M[d, idx[d, j] + 1, j] += frac[d, j]
    A = np.zeros((NDW, LL, NPRED), np.float64)
    w = np.asarray(conv_w, np.float64)
    if mode == "t":
        for p in range(NPRED):
            for k in range(3):
                j = p + k - 1
                if 0 <= j < NPRED:
                    A[:, :, p] += w[0, :, k][:, None] * M[:, :, j]
    else:
        for o in range(NPRED):
            for d in range(NDW):
                for c in range(NPRED):
                    A[d, :, o] += w[o, c, d] * M[d, :, c]
    return A.reshape(NDW * LL, NPRED)


def _plan(offset_t, offset_n, conv_t_w, conv_n_w, W):
    """Returns (order [2*chunk feature indices], chunk, fold_bias, A_t, D)."""
    A_t = _build_A(offset_t, conv_t_w, "t")
    D = _build_A(offset_n, conv_n_w, "n") - A_t
    used = (np.abs(A_t).max(axis=1) > 0) | (np.abs(D).max(axis=1) > 0)
    idx = np.where(used)[0]
    chunk = (len(idx) + 1) // 2
    order = np.concatenate([idx, np.zeros(2 * chunk - len(idx), np.int64)])
    Wf = np.asarray(W, np.float64)
    fold = np.linalg.cond(Wf) < MAX_W_COND
    return order, chunk, fold, A_t, D


def _build_weight_tiles(order, chunk, fold, A_t, D, W):
    """Stationary lhsT tiles [NW, 96, 112]: [0]=blockdiag-W (+bias perm if
    not folded at [NW-1]), [1],[2]=X chunks, [3]=I48."""
    nw = 4 if fold else 5
    wts = np.zeros((nw, 96, 112), np.float64)
    Wf = np.asarray(W, np.float64)
    for j in range(8):
        col = (j // 4) * 64 + (j % 4) * 12
        wts[0, j * 12:(j + 1) * 12, col:col + 12] = Wf
        if not fold:
            for q in range(12):
                wts[4, j * 12 + q, col + q] = 1.0
    wts[3, 0:48, 0:48] = np.eye(48)
    A_u = A_t[order]   # [2*chunk, 12]; padded rows get zero weight below
    D_u = D[order]
    pad = np.ones(2 * chunk)
    pad[len(np.where((np.abs(A_t).max(1) > 0) | (np.abs(D).max(1) > 0))[0]):] = 0
    A_u = A_u * pad[:, None]
    D_u = D_u * pad[:, None]
    for c in range(2):
        for g in range(4):
            rows = slice(g * chunk, (g + 1) * chunk)
            wts[1 + c, rows, g * 12:(g + 1) * 12] = A_u[c * chunk:(c + 1) * chunk]
            wts[1 + c, rows, 64 + g * 12:64 + (g + 1) * 12] = D_u[c * chunk:(c + 1) * chunk]
    return wts.astype(np.float32)


def build_program(chunk=20, fold=True):
    import concourse.bass as bass
    import concourse.tile as tile
    from concourse import bacc, mybir
    from concourse.bass_interp import get_hw_module

    dt_mm = mybir.dt.float16 if STREAM_FP16 else mybir.dt.float32
    f32 = mybir.dt.float32
    dt_out = mybir.dt.float16 if OUT_FP16 else f32
    NW = 4 if fold else 5
    CP = 4 * chunk              # contract partitions for the x path

    nc = bacc.Bacc("TRN2", target_bir_lowering=False, debug=False,
                   num_devices=N_CORES)
    xp = nc.dram_tensor("xp", [BPC, 2, 2, CP, 512], dt_mm, kind="ExternalInput").ap()
    ct = nc.dram_tensor("ct", [BPC, 96, 512], dt_mm, kind="ExternalInput").ap()
    wts = nc.dram_tensor("wts", [NW, 96, 112], dt_mm, kind="ExternalInput").ap()
    if not fold:
        bias = nc.dram_tensor("bias", [96, 512], dt_mm, kind="ExternalInput").ap()
    yp = nc.dram_tensor("yp", [BPC, 96, 512], dt_out, kind="ExternalOutput").ap()
    n_dma_units = sum(e == "dma" for e in ADD_ENGINES)
    if n_dma_units:
        yq = nc.dram_tensor("yq", [8, n_dma_units, 48, 512], f32,
                            kind="ExternalOutput").ap()

    with tile.TileContext(nc) as tc, ExitStack() as ctx:
        consts = ctx.enter_context(tc.tile_pool(name="consts", bufs=1))
        xpool = ctx.enter_context(tc.tile_pool(name="xp", bufs=1))
        cpool = ctx.enter_context(tc.tile_pool(name="ct", bufs=1))
        spool = ctx.enter_context(tc.tile_pool(name="sig", bufs=6))
        tpool = ctx.enter_context(tc.tile_pool(name="tmp", bufs=10))
        opool = ctx.enter_context(tc.tile_pool(name="ost", bufs=1))
        gps = ctx.enter_context(
            tc.tile_pool(name="gps", bufs=2, space=bass.MemorySpace.PSUM))
        xps = ctx.enter_context(
            tc.tile_pool(name="xps", bufs=6, space=bass.MemorySpace.PSUM))

        # ---- all input DMAs up-front, in pipeline order, no ring waits ----
        w_sb = consts.tile([96, NW * 112], dt_mm)
        nc.sync.dma_start(
            w_sb[:].rearrange("p (n f) -> p n f", n=NW),
            wts[:].rearrange("n p f -> p n f"),
        )
        if not fold:
            bias_sb = consts.tile([96, 512], dt_mm)
            nc.sync.dma_start(bias_sb[:], bias[:])

        ct_tiles = []
        xp_tiles = [None] * 8   # per pair: tile or (tile, tile) when fine

        def make_ct(i4):
            ct_sb = cpool.tile([96, 4 * 512], dt_mm, name=f"ct{i4}")
            ct_tiles.append(ct_sb)
            return ct_sb

        def load_ct_half(i4, half):
            # 2-supertile ct pieces: pair (2*i4+half)'s gate data lands just
            # before its xp pair, halving the gate-path fill latency
            c0 = i4 * 4 + half * 2
            nc.sync.dma_start(
                ct_tiles[i4][:, half * 1024:(half + 1) * 1024]
                .rearrange("p (b f) -> p b f", b=2),
                ct[c0:c0 + 2].rearrange("b p f -> p b f"),
            )

        def load_xp(pair):
            b0 = pair * 2
            if pair in FINE_XP_PAIRS:
                tl = []
                for bi in range(2):
                    t1 = xpool.tile([CP, 4 * 512], dt_mm, name=f"xf{pair}_{bi}")
                    nc.sync.dma_start(
                        t1[:].rearrange("p (h c f) -> p h c f", h=2, c=2),
                        xp[b0 + bi].rearrange("h c p f -> p h c f"),
                    )
                    tl.append(t1)
                xp_tiles[pair] = tuple(tl)
            else:
                t2 = xpool.tile([CP, 2 * 4 * 512], dt_mm, name=f"xp{pair}")
                nc.sync.dma_start(
                    t2[:].rearrange("p (b h c f) -> p b h c f", b=2, h=2, c=2),
                    xp[b0:b0 + 2].rearrange("b h c p f -> p b h c f"),
                )
                xp_tiles[pair] = t2

        for i4 in range(4):
            make_ct(i4)
        for pair in range(8):
            load_ct_half(pair // 2, pair % 2)
            load_xp(pair)

        def w_slice(k, width=112):
            return w_sb[:, k * 112:k * 112 + width]

        if N_WARM_MM:
            # tiny warm-up matmuls: near-free, but bump the PE out of its
            # cold p-state before the first real gate matmul
            wp = xps.tile([112, 512], f32, name="warmps", tag="px")
            for i in range(N_WARM_MM):
                nc.tensor.matmul(wp[0:16, 0:16], w_sb[:, 0:16],
                                 w_sb[:, 0:16], start=True, stop=True)

        # ---- compute: gate/sigmoid stage runs one pair ahead of x/combine ----
        def gates_stage(pair):
            i4, half = divmod(pair, 2)
            ct_sb = ct_tiles[i4]
            g_list = []
            for bi in range(2):
                bb = half * 2 + bi
                g_ps = gps.tile([112, 512], f32)
                nc.tensor.matmul(g_ps[:], w_slice(0),
                                 ct_sb[:, bb * 512:(bb + 1) * 512],
                                 start=True, stop=fold)
                g_list.append(g_ps)
            if not fold:
                for bi in range(2):
                    nc.tensor.matmul(g_list[bi][:], w_slice(NW - 1), bias_sb[:],
                                     start=False, stop=True)
            s_list = []
            for bi in range(2):
                s_sb = spool.tile([112, 512], f32)
                nc.scalar.activation(
                    s_sb[:], g_list[bi][:],
                    mybir.ActivationFunctionType.Sigmoid)
                s_list.append(s_sb)
            return s_list

        def x_combine_stage(pair, s_list):
            b0 = pair * 2
            o_sb = opool.tile([112, 2 * 512], dt_out, name=f"o{pair}")
            # x-path matmuls: one pass produces [Pt | pad | Pd] per (bi, h)
            px_list = [xps.tile([112, 512], f32, name=f"px{k}", tag="px")
                       for k in range(4)]
            for bi in range(2):
                for ci in range(2):
                    for h in range(2):
                        k = bi * 2 + h
                        if pair in FINE_XP_PAIRS:
                            rhs = xp_tiles[pair][bi][:, h * 1024 + ci * 512:
                                                     h * 1024 + (ci + 1) * 512]
                        else:
                            rhs = xp_tiles[pair][:, k * 1024 + ci * 512:
                                                 k * 1024 + (ci + 1) * 512]
                        nc.tensor.matmul(px_list[k][:],
                                         w_sb[0:CP, (1 + ci) * 112:(2 + ci) * 112],
                                         rhs, start=(ci == 0), stop=(ci == 1))
            # combine: o = Pt + Pd * S, spread across DVE / PE+ACT / PSUM-DMA
            out_eng = getattr(nc, OUT_DMA_ENGINE)
            dma_j = 0
            for k in range(4):
                bi, h = divmod(k, 2)
                px = px_list[k]
                s_sb = s_list[bi]
                o_slice = o_sb[64 * h:64 * h + 48, bi * 512:(bi + 1) * 512]
                t_sb = tpool.tile([48, 512], dt_mm)
                nc.vector.tensor_mul(
                    t_sb[:], px[64:112, :], s_sb[64 * h:64 * h + 48, :])
                eng = ADD_ENGINES[k]
                if eng in ("pe", "dma"):
                    # out = Pt + T via identity-matmul accumulate on PE
                    nc.tensor.matmul(px[0:48, :],
                                     w_sb[0:48, 3 * 112:3 * 112 + 48],
                                     t_sb[:], start=False, stop=True,
                                     skip_group_check=True)
                    if eng == "pe":
                        nc.scalar.activation(
                            o_slice, px[0:48, :],
                            mybir.ActivationFunctionType.Copy)
                    else:
                        # store the finished rows straight from PSUM (f32)
                        out_eng.dma_start(yq[pair, dma_j], px[0:48, :])
                        dma_j += 1
                else:
                    nc.vector.tensor_add(o_slice, px[0:48, :], t_sb[:])
            # fp16 output stores (compute-gated)
            b0 = pair * 2
            if SPLIT_LAST_OUT and pair == 7:
                for h in range(2):
                    for bi in range(2):
                        out_eng.dma_start(
                            yp[b0 + bi, 48 * h:48 * h + 48],
                            o_sb[64 * h:64 * h + 48, bi * 512:(bi + 1) * 512])
            else:
                out_eng.dma_start(
                    yp[b0:b0 + 2, 0:48].rearrange("b p f -> p b f"),
                    o_sb[0:48, :].rearrange("p (b f) -> p b f", b=2))
                out_eng.dma_start(
                    yp[b0:b0 + 2, 48:96].rearrange("b p f -> p b f"),
                    o_sb[64:112, :].rearrange("p (b f) -> p b f", b=2))

        s_store = {}
        for p in range(9):
            if p < 8:
                s_store[p] = gates_stage(p)
            if p >= 1:
                x_combine_stage(p - 1, s_store.pop(p - 1))

    nc.compile()
    nc.m = get_hw_module(nc.m)
    return nc


_PROGRAMS = {}


def _get_program(chunk, fold):
    key = (chunk, fold)
    if key not in _PROGRAMS:
        _PROGRAMS[key] = build_program(chunk, fold)
    return _PROGRAMS[key]


def pack_inputs(inp, ctrl, bparam, W, order, chunk, fold):
    """Host-side shard + layout packing. Returns in_maps (list of 8 dicts)."""
    X = np.asarray(inp, np.float32).reshape(B, 2, 4, 512, NDW * LL)
    Xu = X[..., order].reshape(B, 2, 4, 512, 2, chunk)
    Xpack = np.ascontiguousarray(Xu.transpose(0, 1, 4, 2, 5, 3)).reshape(
        B, 2, 2, 4 * chunk, 512)
    ctf = np.asarray(ctrl, np.float64)
    if fold:
        binv = np.asarray(bparam, np.float64) @ np.linalg.inv(
            np.asarray(W, np.float64))                      # [NNODES, 12]
        ctf = ctf + binv[None, :, :]
    CT = np.ascontiguousarray(
        ctf.astype(np.float32).reshape(B, NG, 512, 12).transpose(0, 1, 3, 2)
    ).reshape(B, 96, 512)
    dt = np.float16 if STREAM_FP16 else np.float32
    Xpack = Xpack.astype(dt)
    CT = CT.astype(dt)
    in_maps = []
    for c in range(N_CORES):
        sl = slice(c * BPC, (c + 1) * BPC)
        in_maps.append({"xp": Xpack[sl], "ct": CT[sl]})
    return in_maps


def unpack_output(results):
    """results: list of 8 dicts with 'yp' [BPC, 96, 512] (+ 'yq' f32 direct
    slices) -> out [B, N, 12]."""
    yp = np.concatenate([r["yp"].astype(np.float32) for r in results], axis=0)
    return np.ascontiguousarray(
        yp.reshape(B, NG, 12, 512).transpose(0, 1, 3, 2)
    ).reshape(B, NNODES, NPRED)


def kernel(inp, ctrl, offset_t, offset_n, conv_t_w, conv_t_b, conv_n_w,
           conv_n_b, W, bparam):
    from concourse.bass_utils import run_bass_kernel_spmd

    order, chunk, fold, A_t, D = _plan(offset_t, offset_n, conv_t_w,
                                       conv_n_w, W)
    nc = _get_program(chunk, fold)
    wts_np = _build_weight_tiles(order, chunk, fold, A_t, D, W)
    dt = np.float16 if STREAM_FP16 else np.float32
    in_maps = pack_inputs(inp, ctrl, bparam, W, order, chunk, fold)
    for m in in_maps:
        m["wts"] = wts_np.astype(dt)
        if not fold:
            bias_t = np.ascontiguousarray(
                np.asarray(bparam, np.float32).reshape(NG, 512, 12)
                .transpose(0, 2, 1)).reshape(96, 512)
            m["bias"] = bias_t.astype(dt)
    res = run_bass_kernel_spmd(nc, in_maps, core_ids=list(range(N_CORES)))
    out = unpack_output(res.results)
    # Conv biases are zeros in this module's init, so the device kernel omits
    # them. If ever nonzero, apply the exact correction on the host.
    ctb = float(np.asarray(conv_t_b).reshape(-1)[0])
    cnb = np.asarray(conv_n_b, np.float32)
    if ctb != 0.0 or np.any(cnb != 0.0):
        G = np.asarray(ctrl, np.float32).reshape(B * NNODES, NPRED) @ np.asarray(
            W, np.float32)
        G += np.tile(np.asarray(bparam, np.float32), (B, 1))
        S = 1.0 / (1.0 + np.exp(-G))
        out = out + (ctb + (cnb[None, :] - ctb) * S).reshape(B, NNODES, NPRED)
    return out.astype(np.float32)
